# revision 6
# baseline (speedup 1.0000x reference)
"""Causal multi-head attention on 8 TRN2 NeuronCores.

Problem: x[4, 2048, 2048] @ Wq/Wk/Wv[2048, 2048] -> 16-head causal attention
(head_dim 128) -> out-proj Wo[2048, 2048] + b_out.

Sharding: 4-way head tensor-parallel x 2-way batch data-parallel.
Core c handles head group (c % 4) (4 heads = 512 cols of Wq/Wk/Wv, 512 rows
of Wo) and batch pair (c // 4). Each core emits a partial out-projection for
its 2 batches; the host sums the 4 partials per batch pair (the "all-reduce")
and adds the bias.

Per-core pipeline (bf16 matmul operands, fp32 PSUM accumulation):
  P1: cast x to bf16, PE-transpose to xT, project to qT/kT [d, s] and
      v [s, d]; stage to DRAM.
  P2: per (batch, head): scoresT[sk, sq] = kT.T @ qT (one matmul per tile,
      K = head_dim = 128), exp via ScalarE with 1/sqrt(128) folded into the
      activation pre-scale (no max subtraction: |scores| <= ~5), causal mask
      via affine_select on diagonal tiles only (sub-diagonal tiles skipped),
      denominator via ones-vector matmul, ctxT accumulated in PSUM and
      normalized by the softmax reciprocal on the way out.
  P3: out-proj y = ctxT.T @ Wo per batch; DMA partial result.
"""

import math

import numpy as np

P = 128
S = 2048          # sequence length
D = 2048          # model dim
NB = 2            # batches per core
SL = NB * S       # local rows (4096)
DL = 512          # local head dims (4 heads x 128)
HL = 4            # local heads
NI = D // P       # 16 i-tiles
SCHUNK = 512
NCHUNK = SL // SCHUNK  # 8
SCALE = 1.0 / math.sqrt(128.0)
N_CORES = 8

_CACHE = {}


def _split_multi_waits(nc):
    """This walrus build accepts at most ONE sync-wait per instruction
    (setupSyncWait: 'Too many sync wait commands'), but Tile emits up to
    ~3 waits per instruction and the kernel-tail drain carries one wait per
    outstanding semaphore. Hoist excess waits onto single-wait nops inserted
    immediately before the instruction on the same engine stream."""
    import bass_rust

    SyncInfo = bass_rust.SyncInfo
    n = 0
    for f in nc.m.functions:
        for b in f.blocks:
            out = []
            changed = False
            for inst in list(b.instructions):
                si = getattr(inst, "sync_info", None)
                if si is not None and si.on_wait and len(si.on_wait) > 1:
                    waits = list(si.on_wait)
                    for w in waits[:-1]:
                        n += 1
                        nop = bass_rust.InstNoOp(
                            name=f"waitsplit-{n}", ins=[], outs=[]
                        )
                        nop.engine = inst.engine
                        nop.sync_info = SyncInfo(on_wait=[w], on_update=[])
                        out.append(nop)
                    inst.sync_info = SyncInfo(
                        on_wait=[waits[-1]], on_update=list(si.on_update or [])
                    )
                    changed = True
                out.append(inst)
            if changed:
                b.instructions = out


def _build():
    import concourse.bass as bass
    import concourse.mybir as mybir
    import concourse.tile as tile
    from concourse.masks import make_identity

    f32 = mybir.dt.float32
    bf16 = mybir.dt.bfloat16
    Exp = mybir.ActivationFunctionType.Exp

    nc = bass.Bass()
    x_in = nc.declare_dram_parameter("x", [SL, D], f32, isOutput=False)
    wq_in = nc.declare_dram_parameter("wq", [D, DL], f32, isOutput=False)
    wk_in = nc.declare_dram_parameter("wk", [D, DL], f32, isOutput=False)
    wv_in = nc.declare_dram_parameter("wv", [D, DL], f32, isOutput=False)
    wo_in = nc.declare_dram_parameter("wo", [DL, D], f32, isOutput=False)
    y_out = nc.declare_dram_parameter("y", [SL, D], f32, isOutput=True)

    with tile.TileContext(nc) as tc:
        _emit(nc, tc, mybir, make_identity, x_in, wq_in, wk_in, wv_in, wo_in, y_out)
    _split_multi_waits(nc)
    return nc


def _emit(nc, tc, mybir, make_identity, x_in, wq_in, wk_in, wv_in, wo_in, y_out):
    from contextlib import ExitStack

    f32 = mybir.dt.float32
    bf16 = mybir.dt.bfloat16
    Exp = mybir.ActivationFunctionType.Exp

    ctx = ExitStack()
    with ctx:
        dram = ctx.enter_context(tc.tile_pool(name="dram", bufs=1, space="DRAM"))
        consts = ctx.enter_context(tc.tile_pool(name="consts", bufs=1))
        wpool = ctx.enter_context(tc.tile_pool(name="wpool", bufs=1))
        xin_pool = ctx.enter_context(tc.tile_pool(name="xin_pool", bufs=2))
        xbf_pool = ctx.enter_context(tc.tile_pool(name="xbf_pool", bufs=3))
        xt_pool = ctx.enter_context(tc.tile_pool(name="xt_pool", bufs=2))
        qkv_pool = ctx.enter_context(tc.tile_pool(name="qkv_pool", bufs=4))
        att_pool = ctx.enter_context(tc.tile_pool(name="att_pool", bufs=2))
        out_pool = ctx.enter_context(tc.tile_pool(name="out_pool", bufs=3))
        pbig = ctx.enter_context(tc.tile_pool(name="pbig", bufs=2, space="PSUM"))
        psmall = ctx.enter_context(tc.tile_pool(name="psmall", bufs=4, space="PSUM"))

        # DRAM staging for q/k/v (transposed layouts) and ctx
        import concourse.bass as bass

        qT_d = dram.tile([DL, SL], bf16, name="qT_d")
        kT_d = dram.tile([DL, SL], bf16, name="kT_d")
        v_d = dram.tile([SL, DL], bf16, name="v_d")
        cT_d = dram.tile([DL, SL], bf16, name="cT_d")

        qT_r = qT_d.rearrange("(a p) s -> p a s", p=P)   # [128, 4, 4096]
        kT_r = kT_d.rearrange("(a p) s -> p a s", p=P)
        v_r = v_d.rearrange("(n p) d -> p n d", p=P)     # [128, 32, 512]
        cT_r = cT_d.rearrange("(a p) s -> p a s", p=P)

        ident = consts.tile([P, P], bf16, name="ident")
        make_identity(nc, ident)
        ones = consts.tile([P, 1], bf16, name="ones")
        nc.vector.memset(ones, 1.0)

        # --- weights: DMA f32, cast to bf16 ---
        wq_sb = wpool.tile([P, NI, DL], bf16, name="wq_sb")
        wk_sb = wpool.tile([P, NI, DL], bf16, name="wk_sb")
        wv_sb = wpool.tile([P, NI, DL], bf16, name="wv_sb")
        wo_sb = wpool.tile([P, HL, D], bf16, name="wo_sb")
        for w_in, w_sb in ((wq_in, wq_sb), (wk_in, wk_sb), (wv_in, wv_sb)):
            for i in range(NI):
                wt = xin_pool.tile([P, D], f32, name="wt", tag="xin")
                nc.sync.dma_start(out=wt[:, :DL], in_=w_in[P * i : P * (i + 1), :])
                nc.vector.tensor_copy(w_sb[:, i, :], wt[:, :DL])
        for dt in range(HL):
            wt = xin_pool.tile([P, D], f32, name="wt2", tag="xin")
            nc.sync.dma_start(out=wt, in_=wo_in[P * dt : P * (dt + 1), :])
            nc.vector.tensor_copy(wo_sb[:, dt, :], wt)

        # --- P1: transpose x, project to qT/kT/v ---
        for ch in range(NCHUNK):
            xT = xt_pool.tile([P, NI, SCHUNK], bf16, name="xT", tag="xT")
            for st in range(SCHUNK // P):  # 4 s-tiles per chunk
                row0 = SCHUNK * ch + P * st
                xin = xin_pool.tile([P, D], f32, name="xin", tag="xin")
                nc.sync.dma_start(out=xin, in_=x_in[row0 : row0 + P, :])
                xbf = xbf_pool.tile([P, D], bf16, name="xbf", tag="xbf")
                nc.vector.tensor_copy(xbf, xin)
                for ip in range(4):  # pack 4 i-tile transposes per psum tile
                    tp = psmall.tile([P, 512], bf16, name="tp", tag="ps")
                    for k in range(4):
                        i = 4 * ip + k
                        nc.tensor.transpose(
                            tp[:, P * k : P * (k + 1)],
                            xbf[:, P * i : P * (i + 1)],
                            ident,
                        )
                    nc.vector.tensor_copy(
                        xT[:, 4 * ip : 4 * ip + 4, P * st : P * (st + 1)],
                        tp.rearrange("p (a b) -> p a b", a=4),
                    )
            # qT / kT: [d, s] layout; lhsT = W block, rhs = xT
            for w_sb, out_r in ((wq_sb, qT_r), (wk_sb, kT_r)):
                for hp in range(2):  # pairs of head-tiles for wide copies
                    pq = pbig.tile([P, 1024], f32, name="pq", tag="pb")
                    for h2 in range(2):
                        h = 2 * hp + h2
                        for i in range(NI):
                            nc.tensor.matmul(
                                pq[:, 512 * h2 : 512 * (h2 + 1)],
                                lhsT=w_sb[:, i, P * h : P * (h + 1)],
                                rhs=xT[:, i, :],
                                start=(i == 0),
                                stop=(i == NI - 1),
                            )
                    qsb = qkv_pool.tile([P, 1024], bf16, name="qsb", tag="qsb")
                    nc.scalar.copy(qsb, pq)
                    nc.sync.dma_start(
                        out=out_r[
                            :, 2 * hp : 2 * hp + 2, SCHUNK * ch : SCHUNK * (ch + 1)
                        ],
                        in_=qsb.rearrange("p (a b) -> p a b", a=2),
                    )
            # v: [s, d] layout; lhsT = xT block, rhs = Wv
            for sp in range(2):  # pairs of s-tiles
                pv = pbig.tile([P, 1024], f32, name="pv", tag="pb")
                for s2 in range(2):
                    st = 2 * sp + s2
                    for i in range(NI):
                        nc.tensor.matmul(
                            pv[:, 512 * s2 : 512 * (s2 + 1)],
                            lhsT=xT[:, i, P * st : P * (st + 1)],
                            rhs=wv_sb[:, i, :],
                            start=(i == 0),
                            stop=(i == NI - 1),
                        )
                vsb = qkv_pool.tile([P, 1024], bf16, name="vsb", tag="qsb")
                nc.scalar.copy(vsb, pv)
                n0 = 4 * ch + 2 * sp
                nc.sync.dma_start(
                    out=v_r[:, n0 : n0 + 2, :],
                    in_=vsb.rearrange("p (a b) -> p a b", a=2),
                )

        # --- P2: causal attention per (batch, head) ---
        for b in range(NB):
            for h in range(HL):
                ktb = att_pool.tile([P, S], bf16, name="ktb", tag="ktb")
                nc.sync.dma_start(
                    out=ktb, in_=kT_d[P * h : P * (h + 1), S * b : S * (b + 1)]
                )
                vtb = att_pool.tile([P, S // P, P], bf16, name="vtb", tag="vtb")
                nc.sync.dma_start(
                    out=vtb,
                    in_=v_r[:, (S // P) * b : (S // P) * (b + 1), P * h : P * (h + 1)],
                )
                for c in range(S // SCHUNK):  # 4 sq-chunks
                    qtc = att_pool.tile([P, SCHUNK], bf16, name="qtc", tag="qtc", bufs=3)
                    nc.sync.dma_start(
                        out=qtc,
                        in_=qT_d[
                            P * h : P * (h + 1),
                            S * b + SCHUNK * c : S * b + SCHUNK * (c + 1),
                        ],
                    )
                    pctx = psmall.tile([P, 512], f32, name="pctx", tag="ps")
                    pden = psmall.tile([P, 512], f32, name="pden", tag="ps")
                    jmax = 4 * c + 4  # sk-tiles with sk_start <= sq_end
                    for jp in range(jmax // 2):
                        j0 = 2 * jp
                        ps2 = pbig.tile([P, 1024], f32, name="ps2", tag="pb")
                        for j2 in range(2):
                            j = j0 + j2
                            nc.tensor.matmul(
                                ps2[:, 512 * j2 : 512 * (j2 + 1)],
                                lhsT=ktb[:, P * j : P * (j + 1)],
                                rhs=qtc,
                                start=True,
                                stop=True,
                            )
                        at2 = att_pool.tile([P, 1024], bf16, name="at2", tag="at2", bufs=3)
                        nc.scalar.activation(at2, ps2, Exp, scale=SCALE)
                        if j0 >= 4 * c:  # diagonal pair: zero sk > sq
                            nc.gpsimd.affine_select(
                                out=at2.rearrange("p (a b) -> p a b", a=2),
                                in_=at2.rearrange("p (a b) -> p a b", a=2),
                                compare_op=mybir.AluOpType.is_ge,
                                fill=0.0,
                                base=SCHUNK * c - P * j0,
                                channel_multiplier=-1,
                                pattern=[[-P, 2], [1, SCHUNK]],
                            )
                        for j2 in range(2):
                            j = j0 + j2
                            a_sl = at2[:, 512 * j2 : 512 * (j2 + 1)]
                            nc.tensor.matmul(
                                pctx,
                                lhsT=vtb[:, j, :],
                                rhs=a_sl,
                                start=(j == 0),
                                stop=(j == jmax - 1),
                            )
                            nc.tensor.matmul(
                                pden[:1, :],
                                lhsT=ones,
                                rhs=a_sl,
                                start=(j == 0),
                                stop=(j == jmax - 1),
                            )
                    rec = att_pool.tile([1, 512], f32, name="rec", tag="rec", bufs=3)
                    nc.vector.reciprocal(rec, pden[:1, :])
                    # broadcast [1, 512] -> [128, 512] via DRAM roundtrip
                    # (stride-0 partition reads are only legal on the DRAM side)
                    rb = dram.tile([512], f32, name="rb", tag="rb", bufs=3)
                    nc.sync.dma_start(out=rb, in_=rec)
                    rb_bc = bass.AP(rb.tensor, rb.offset, [[0, P], [1, 512]])
                    recb = att_pool.tile([P, 512], f32, name="recb", tag="recb", bufs=3)
                    nc.sync.dma_start(out=recb, in_=rb_bc)
                    csb = att_pool.tile([P, 512], bf16, name="csb", tag="csb", bufs=3)
                    nc.vector.tensor_mul(csb, pctx, recb)
                    nc.sync.dma_start(
                        out=cT_d[
                            P * h : P * (h + 1),
                            S * b + SCHUNK * c : S * b + SCHUNK * (c + 1),
                        ],
                        in_=csb,
                    )

        # --- P3: out-projection ---
        for b in range(NB):
            for t in range(S // P):  # 16 sq-tiles
                col0 = S * b + P * t
                ctb = out_pool.tile([P, HL, P], bf16, name="ctb", tag="ctb")
                nc.sync.dma_start(out=ctb, in_=cT_r[:, :, col0 : col0 + P])
                for fp in range(2):  # pairs of f-chunks
                    py = pbig.tile([P, 1024], f32, name="py", tag="pb")
                    for f2 in range(2):
                        f = 2 * fp + f2
                        for dt in range(HL):
                            nc.tensor.matmul(
                                py[:, 512 * f2 : 512 * (f2 + 1)],
                                lhsT=ctb[:, dt, :],
                                rhs=wo_sb[:, dt, 512 * f : 512 * (f + 1)],
                                start=(dt == 0),
                                stop=(dt == HL - 1),
                            )
                    ysb = out_pool.tile([P, 1024], f32, name="ysb", tag="ysb")
                    nc.scalar.copy(ysb, py)
                    nc.sync.dma_start(
                        out=y_out[col0 : col0 + P, 1024 * fp : 1024 * (fp + 1)],
                        in_=ysb,
                    )


def _get_nc():
    if "nc" not in _CACHE:
        _CACHE["nc"] = _build()
    return _CACHE["nc"]


def _run(inputs, trace=False):
    from concourse.bass_utils import run_bass_kernel_spmd

    x = np.ascontiguousarray(np.asarray(inputs["x"], dtype=np.float32))
    wq = np.asarray(inputs["W_query"], dtype=np.float32)
    wk = np.asarray(inputs["W_key"], dtype=np.float32)
    wv = np.asarray(inputs["W_value"], dtype=np.float32)
    wo = np.asarray(inputs["W_out"], dtype=np.float32)
    b_out = np.asarray(inputs["b_out"], dtype=np.float32)

    xf = x.reshape(2, SL, D)  # batch pairs
    in_maps = []
    for c in range(N_CORES):
        pair = c // 4
        hg = c % 4
        in_maps.append(
            {
                "x": np.ascontiguousarray(xf[pair]),
                "wq": np.ascontiguousarray(wq[:, DL * hg : DL * (hg + 1)]),
                "wk": np.ascontiguousarray(wk[:, DL * hg : DL * (hg + 1)]),
                "wv": np.ascontiguousarray(wv[:, DL * hg : DL * (hg + 1)]),
                "wo": np.ascontiguousarray(wo[DL * hg : DL * (hg + 1), :]),
            }
        )

    nc = _get_nc()
    res = run_bass_kernel_spmd(nc, in_maps, core_ids=list(range(N_CORES)), trace=trace)

    y = np.zeros((2, SL, D), dtype=np.float32)
    for c in range(N_CORES):
        y[c // 4] += res.results[c]["y"]
    y += b_out[None, None, :]
    out = y.reshape(4, S, D)
    return out, res


def kernel(**inputs) -> np.ndarray:
    out, _ = _run(inputs, trace=False)
    return out


# revision 11
# speedup vs baseline: 1.1417x; 1.1417x over previous
"""Causal multi-head attention on 8 TRN2 NeuronCores.

Problem: x[4, 2048, 2048] @ Wq/Wk/Wv[2048, 2048] -> 16-head causal attention
(head_dim 128) -> out-proj Wo[2048, 2048] + b_out.

Sharding: 4-way head tensor-parallel x 2-way batch data-parallel.
Core c handles head group (c % 4) (4 heads = 512 cols of Wq/Wk/Wv, 512 rows
of Wo) and batch pair (c // 4). Each core emits a partial out-projection for
its 2 batches; the host sums the 4 partials per batch pair (the "all-reduce")
and adds the bias.

Per-core pipeline (bf16 matmul operands, fp32 PSUM accumulation):
  P1: cast x to bf16, PE-transpose to xT, project to qT/kT [d, s] and
      v [s, d]; stage to DRAM.
  P2: per (batch, head): scoresT[sk, sq] = kT.T @ qT (one matmul per tile,
      K = head_dim = 128), exp via ScalarE with 1/sqrt(128) folded into the
      activation pre-scale (no max subtraction: |scores| <= ~5), causal mask
      via affine_select on diagonal tiles only (sub-diagonal tiles skipped),
      denominator via ones-vector matmul, ctxT accumulated in PSUM and
      normalized by the softmax reciprocal on the way out.
  P3: out-proj y = ctxT.T @ Wo per batch; DMA partial result.
"""

import math

import numpy as np

P = 128
S = 2048          # sequence length
D = 2048          # model dim
NB = 2            # batches per core
SL = NB * S       # local rows (4096)
DL = 512          # local head dims (4 heads x 128)
HL = 4            # local heads
NI = D // P       # 16 i-tiles
SCHUNK = 512
NCHUNK = SL // SCHUNK  # 8
SCALE = 1.0 / math.sqrt(128.0)
N_CORES = 8

_CACHE = {}


def _split_multi_waits(nc):
    """This walrus build accepts at most ONE sync-wait per instruction
    (setupSyncWait: 'Too many sync wait commands'), but Tile emits up to
    ~3 waits per instruction and the kernel-tail drain carries one wait per
    outstanding semaphore. Hoist excess waits onto single-wait nops inserted
    immediately before the instruction on the same engine stream."""
    import bass_rust

    SyncInfo = bass_rust.SyncInfo
    n = 0
    for f in nc.m.functions:
        for b in f.blocks:
            out = []
            changed = False
            for inst in list(b.instructions):
                si = getattr(inst, "sync_info", None)
                if si is not None and si.on_wait and len(si.on_wait) > 1:
                    waits = list(si.on_wait)
                    for w in waits[:-1]:
                        n += 1
                        nop = bass_rust.InstNoOp(
                            name=f"waitsplit-{n}", ins=[], outs=[]
                        )
                        nop.engine = inst.engine
                        nop.sync_info = SyncInfo(on_wait=[w], on_update=[])
                        out.append(nop)
                    inst.sync_info = SyncInfo(
                        on_wait=[waits[-1]], on_update=list(si.on_update or [])
                    )
                    changed = True
                out.append(inst)
            if changed:
                b.instructions = out


def _build():
    import concourse.bass as bass
    import concourse.mybir as mybir
    import concourse.tile as tile
    from concourse.masks import make_identity

    f32 = mybir.dt.float32
    bf16 = mybir.dt.bfloat16
    Exp = mybir.ActivationFunctionType.Exp

    nc = bass.Bass()
    x_in = nc.declare_dram_parameter("x", [SL, D], f32, isOutput=False)
    wq_in = nc.declare_dram_parameter("wq", [D, DL], f32, isOutput=False)
    wk_in = nc.declare_dram_parameter("wk", [D, DL], f32, isOutput=False)
    wv_in = nc.declare_dram_parameter("wv", [D, DL], f32, isOutput=False)
    wo_in = nc.declare_dram_parameter("wo", [DL, D], f32, isOutput=False)
    y_out = nc.declare_dram_parameter("y", [SL, D], f32, isOutput=True)

    with tile.TileContext(nc) as tc:
        _emit(nc, tc, mybir, make_identity, x_in, wq_in, wk_in, wv_in, wo_in, y_out)
    _split_multi_waits(nc)
    return nc


def _emit(nc, tc, mybir, make_identity, x_in, wq_in, wk_in, wv_in, wo_in, y_out):
    from contextlib import ExitStack

    f32 = mybir.dt.float32
    bf16 = mybir.dt.bfloat16
    Exp = mybir.ActivationFunctionType.Exp

    ctx = ExitStack()
    with ctx:
        dram = ctx.enter_context(tc.tile_pool(name="dram", bufs=1, space="DRAM"))
        consts = ctx.enter_context(tc.tile_pool(name="consts", bufs=1))
        wpool = ctx.enter_context(tc.tile_pool(name="wpool", bufs=1))
        xin_pool = ctx.enter_context(tc.tile_pool(name="xin_pool", bufs=2))
        xbf_pool = ctx.enter_context(tc.tile_pool(name="xbf_pool", bufs=2))
        xt_pool = ctx.enter_context(tc.tile_pool(name="xt_pool", bufs=2))
        qkv_pool = ctx.enter_context(tc.tile_pool(name="qkv_pool", bufs=4))
        att_pool = ctx.enter_context(tc.tile_pool(name="att_pool", bufs=2))
        out_pool = ctx.enter_context(tc.tile_pool(name="out_pool", bufs=3))
        pbig = ctx.enter_context(tc.tile_pool(name="pbig", bufs=2, space="PSUM"))
        psmall = ctx.enter_context(tc.tile_pool(name="psmall", bufs=4, space="PSUM"))

        # DRAM staging for q/k/v (transposed layouts) and ctx
        import concourse.bass as bass

        qT_d = dram.tile([DL, SL], bf16, name="qT_d")
        kT_d = dram.tile([DL, SL], bf16, name="kT_d")
        v_d = dram.tile([SL, DL], bf16, name="v_d")
        cT_d = dram.tile([DL, SL], bf16, name="cT_d")

        qT_r = qT_d.rearrange("(a p) s -> p a s", p=P)   # [128, 4, 4096]
        kT_r = kT_d.rearrange("(a p) s -> p a s", p=P)
        v_r = v_d.rearrange("(n p) d -> p n d", p=P)     # [128, 32, 512]
        cT_r = cT_d.rearrange("(a p) s -> p a s", p=P)

        ident = consts.tile([P, P], bf16, name="ident")
        make_identity(nc, ident)
        ones = consts.tile([P, 1], bf16, name="ones")
        nc.vector.memset(ones, 1.0)

        # --- weights: batched f32 DMA through a dedicated pool, cast to bf16 ---
        wstg = ctx.enter_context(tc.tile_pool(name="wstg", bufs=2))
        wq_sb = wpool.tile([P, NI, DL], bf16, name="wq_sb")
        wk_sb = wpool.tile([P, NI, DL], bf16, name="wk_sb")
        wv_sb = wpool.tile([P, NI, DL], bf16, name="wv_sb")
        wo_sb = wpool.tile([P, HL, D], bf16, name="wo_sb")

        def emit_weight_loads():
            for w_in, w_sb in ((wq_in, wq_sb), (wk_in, wk_sb), (wv_in, wv_sb)):
                w_r = w_in.rearrange("(a p) d -> p a d", p=P)  # [128, 16, 512]
                for g in range(4):
                    wt = wstg.tile([P, 4, DL], f32, name="wt", tag="wt")
                    nc.sync.dma_start(out=wt, in_=w_r[:, 4 * g : 4 * g + 4, :])
                    nc.vector.tensor_copy(w_sb[:, 4 * g : 4 * g + 4, :], wt)
            for dt in range(HL):
                wt = wstg.tile([P, 4, DL], f32, name="wt2", tag="wt")
                nc.sync.dma_start(
                    out=wt.rearrange("p a d -> p (a d)"),
                    in_=wo_in[P * dt : P * (dt + 1), :],
                )
                nc.vector.tensor_copy(
                    wo_sb[:, dt, :], wt.rearrange("p a d -> p (a d)")
                )

        # --- P1: transpose x, project to qT/kT/v ---
        def load_transpose_chunk(ch):
            xT = xt_pool.tile([P, NI, SCHUNK], bf16, name="xT", tag="xT")
            for st in range(SCHUNK // P):  # 4 s-tiles per chunk
                row0 = SCHUNK * ch + P * st
                xin = xin_pool.tile([P, D], f32, name="xin", tag="xin")
                nc.sync.dma_start(out=xin, in_=x_in[row0 : row0 + P, :])
                xbf = xbf_pool.tile([P, D], bf16, name="xbf", tag="xbf")
                nc.vector.tensor_copy(xbf, xin)
                for ip in range(4):  # pack 4 i-tile transposes per psum tile
                    tp = psmall.tile([P, 512], bf16, name="tp", tag="ps")
                    for k in range(4):
                        i = 4 * ip + k
                        nc.tensor.transpose(
                            tp[:, P * k : P * (k + 1)],
                            xbf[:, P * i : P * (i + 1)],
                            ident,
                        )
                    nc.vector.tensor_copy(
                        xT[:, 4 * ip : 4 * ip + 4, P * st : P * (st + 1)],
                        tp.rearrange("p (a b) -> p a b", a=4),
                    )
            return xT

        xT_next = load_transpose_chunk(0)
        emit_weight_loads()
        for ch in range(NCHUNK):
            xT = xT_next
            if ch + 1 < NCHUNK:
                xT_next = load_transpose_chunk(ch + 1)
            # qT / kT: [d, s] layout; lhsT = W block, rhs = xT
            for w_sb, out_r in ((wq_sb, qT_r), (wk_sb, kT_r)):
                for hp in range(2):  # pairs of head-tiles for wide copies
                    pq = pbig.tile([P, 1024], f32, name="pq", tag="pb")
                    for h2 in range(2):
                        h = 2 * hp + h2
                        for i in range(NI):
                            nc.tensor.matmul(
                                pq[:, 512 * h2 : 512 * (h2 + 1)],
                                lhsT=w_sb[:, i, P * h : P * (h + 1)],
                                rhs=xT[:, i, :],
                                start=(i == 0),
                                stop=(i == NI - 1),
                            )
                    qsb = qkv_pool.tile([P, 1024], bf16, name="qsb", tag="qsb")
                    nc.scalar.copy(qsb, pq)
                    nc.sync.dma_start(
                        out=out_r[
                            :, 2 * hp : 2 * hp + 2, SCHUNK * ch : SCHUNK * (ch + 1)
                        ],
                        in_=qsb.rearrange("p (a b) -> p a b", a=2),
                    )
            # v: [s, d] layout; lhsT = xT block, rhs = Wv
            for sp in range(2):  # pairs of s-tiles
                pv = pbig.tile([P, 1024], f32, name="pv", tag="pb")
                for s2 in range(2):
                    st = 2 * sp + s2
                    for i in range(NI):
                        nc.tensor.matmul(
                            pv[:, 512 * s2 : 512 * (s2 + 1)],
                            lhsT=xT[:, i, P * st : P * (st + 1)],
                            rhs=wv_sb[:, i, :],
                            start=(i == 0),
                            stop=(i == NI - 1),
                        )
                vsb = qkv_pool.tile([P, 1024], bf16, name="vsb", tag="qsb")
                nc.scalar.copy(vsb, pv)
                n0 = 4 * ch + 2 * sp
                nc.sync.dma_start(
                    out=v_r[:, n0 : n0 + 2, :],
                    in_=vsb.rearrange("p (a b) -> p a b", a=2),
                )

        # --- P2: causal attention per (batch, head) ---
        for b in range(NB):
            for h in range(HL):
                ktb = att_pool.tile([P, S], bf16, name="ktb", tag="ktb")
                nc.sync.dma_start(
                    out=ktb, in_=kT_d[P * h : P * (h + 1), S * b : S * (b + 1)]
                )
                vtb = att_pool.tile([P, S // P, P], bf16, name="vtb", tag="vtb")
                nc.sync.dma_start(
                    out=vtb,
                    in_=v_r[:, (S // P) * b : (S // P) * (b + 1), P * h : P * (h + 1)],
                )
                for c in range(S // SCHUNK):  # 4 sq-chunks
                    qtc = att_pool.tile([P, SCHUNK], bf16, name="qtc", tag="qtc", bufs=3)
                    nc.sync.dma_start(
                        out=qtc,
                        in_=qT_d[
                            P * h : P * (h + 1),
                            S * b + SCHUNK * c : S * b + SCHUNK * (c + 1),
                        ],
                    )
                    pctx = psmall.tile([P, 512], f32, name="pctx", tag="ps")
                    pden = psmall.tile([P, 512], f32, name="pden", tag="ps")
                    jmax = 4 * c + 4  # sk-tiles with sk_start <= sq_end
                    for jp in range(jmax // 2):
                        j0 = 2 * jp
                        ps2 = pbig.tile([P, 1024], f32, name="ps2", tag="pb")
                        for j2 in range(2):
                            j = j0 + j2
                            nc.tensor.matmul(
                                ps2[:, 512 * j2 : 512 * (j2 + 1)],
                                lhsT=ktb[:, P * j : P * (j + 1)],
                                rhs=qtc,
                                start=True,
                                stop=True,
                            )
                        at2 = att_pool.tile([P, 1024], bf16, name="at2", tag="at2", bufs=3)
                        nc.scalar.activation(at2, ps2, Exp, scale=SCALE)
                        if j0 >= 4 * c:  # diagonal pair: zero sk > sq
                            nc.gpsimd.affine_select(
                                out=at2.rearrange("p (a b) -> p a b", a=2),
                                in_=at2.rearrange("p (a b) -> p a b", a=2),
                                compare_op=mybir.AluOpType.is_ge,
                                fill=0.0,
                                base=SCHUNK * c - P * j0,
                                channel_multiplier=-1,
                                pattern=[[-P, 2], [1, SCHUNK]],
                            )
                        for j2 in range(2):
                            j = j0 + j2
                            a_sl = at2[:, 512 * j2 : 512 * (j2 + 1)]
                            nc.tensor.matmul(
                                pctx,
                                lhsT=vtb[:, j, :],
                                rhs=a_sl,
                                start=(j == 0),
                                stop=(j == jmax - 1),
                            )
                            nc.tensor.matmul(
                                pden[:1, :],
                                lhsT=ones,
                                rhs=a_sl,
                                start=(j == 0),
                                stop=(j == jmax - 1),
                            )
                    # softmax denominators: copy out of PSUM fast, broadcast
                    # [1, 512] -> [128, 512] via a DRAM roundtrip (stride-0
                    # partition reads are only legal on the DRAM side), THEN
                    # take the reciprocal on all 128 partitions (a
                    # 1-partition reciprocal is ~12x slower).
                    den_sb = att_pool.tile([1, 512], f32, name="den_sb", tag="rec", bufs=3)
                    nc.scalar.copy(den_sb, pden[:1, :])
                    rb = dram.tile([512], f32, name="rb", tag="rb", bufs=3)
                    nc.sync.dma_start(out=rb, in_=den_sb)
                    rb_bc = bass.AP(rb.tensor, rb.offset, [[0, P], [1, 512]])
                    denb = att_pool.tile([P, 512], f32, name="denb", tag="denb", bufs=3)
                    nc.sync.dma_start(out=denb, in_=rb_bc)
                    recb = att_pool.tile([P, 512], f32, name="recb", tag="recb", bufs=3)
                    nc.vector.reciprocal(recb, denb)
                    csb = att_pool.tile([P, 512], bf16, name="csb", tag="csb", bufs=3)
                    nc.vector.tensor_mul(csb, pctx, recb)
                    nc.sync.dma_start(
                        out=cT_d[
                            P * h : P * (h + 1),
                            S * b + SCHUNK * c : S * b + SCHUNK * (c + 1),
                        ],
                        in_=csb,
                    )

        # --- P3: out-projection ---
        for b in range(NB):
            for t in range(S // P):  # 16 sq-tiles
                col0 = S * b + P * t
                ctb = out_pool.tile([P, HL, P], bf16, name="ctb", tag="ctb")
                nc.sync.dma_start(out=ctb, in_=cT_r[:, :, col0 : col0 + P])
                for fp in range(2):  # pairs of f-chunks
                    py = pbig.tile([P, 1024], f32, name="py", tag="pb")
                    for f2 in range(2):
                        f = 2 * fp + f2
                        for dt in range(HL):
                            nc.tensor.matmul(
                                py[:, 512 * f2 : 512 * (f2 + 1)],
                                lhsT=ctb[:, dt, :],
                                rhs=wo_sb[:, dt, 512 * f : 512 * (f + 1)],
                                start=(dt == 0),
                                stop=(dt == HL - 1),
                            )
                    ysb = out_pool.tile([P, 1024], f32, name="ysb", tag="ysb")
                    nc.scalar.copy(ysb, py)
                    nc.sync.dma_start(
                        out=y_out[col0 : col0 + P, 1024 * fp : 1024 * (fp + 1)],
                        in_=ysb,
                    )


def _get_nc():
    if "nc" not in _CACHE:
        _CACHE["nc"] = _build()
    return _CACHE["nc"]


def _run(inputs, trace=False):
    from concourse.bass_utils import run_bass_kernel_spmd

    x = np.ascontiguousarray(np.asarray(inputs["x"], dtype=np.float32))
    wq = np.asarray(inputs["W_query"], dtype=np.float32)
    wk = np.asarray(inputs["W_key"], dtype=np.float32)
    wv = np.asarray(inputs["W_value"], dtype=np.float32)
    wo = np.asarray(inputs["W_out"], dtype=np.float32)
    b_out = np.asarray(inputs["b_out"], dtype=np.float32)

    xf = x.reshape(2, SL, D)  # batch pairs
    in_maps = []
    for c in range(N_CORES):
        pair = c // 4
        hg = c % 4
        in_maps.append(
            {
                "x": np.ascontiguousarray(xf[pair]),
                "wq": np.ascontiguousarray(wq[:, DL * hg : DL * (hg + 1)]),
                "wk": np.ascontiguousarray(wk[:, DL * hg : DL * (hg + 1)]),
                "wv": np.ascontiguousarray(wv[:, DL * hg : DL * (hg + 1)]),
                "wo": np.ascontiguousarray(wo[DL * hg : DL * (hg + 1), :]),
            }
        )

    nc = _get_nc()
    res = run_bass_kernel_spmd(nc, in_maps, core_ids=list(range(N_CORES)), trace=trace)

    y = np.zeros((2, SL, D), dtype=np.float32)
    for c in range(N_CORES):
        y[c // 4] += res.results[c]["y"]
    y += b_out[None, None, :]
    out = y.reshape(4, S, D)
    return out, res


def kernel(**inputs) -> np.ndarray:
    out, _ = _run(inputs, trace=False)
    return out
